# revision 19
# baseline (speedup 1.0000x reference)
"""Bass/Trainium2 kernel for nn_BiencoderRanker: pairwise cosine similarity.

scores[n, m] = <pred_n, cand_m> / (|pred_n| * |cand_m|)
  fp_pred: (1024, 4096) fp32, fp_cand: (16384, 4096) fp32 -> scores (1024, 16384)

Sharding: fp_cand split along M across 8 cores (2048 rows each); fp_pred
replicated. Each core computes the transposed tile scores_t (2048, 1024) in
bf16; the host upcasts/transposes/concatenates.

Host marshalling: inputs are quantized fp32->fp8e4m3 once (rel tol is 2e-2;
measured end-to-end error ~5e-3 incl the bf16 store) and laid out so every
DMA is a full-partition contiguous copy:
  at8[p, kk, i, n]      = predq[n, (2kk+i)*128+p]    K-on-partition (DoubleRow)
  bt8[p, mc, kk, i, m'] = candq[mc*128+m', (2kk+i)*128+p]  (mc-major so per-mc
                          slices land in compute order)
  pn8[p, nb, k]         = predq[nb*128+p, k]         natural, for pred norms

Per-core dataflow (all per-engine queues are strict FIFO — emission order
below is chosen so no queue head ever waits on a late producer):
  PE   : per (mc, kk) one DoubleRow stationary (cand chunk) serves 3 matmuls:
         Gram-diag (cand norms, free), and two 512-wide score tiles
         [128 m, 512 n] accumated over kk. A post-pass dedups the redundant
         per-matmul LDWEIGHTS bass emits (3x fewer weight loads; this is the
         difference between ~525us and ~140us measured).
  ACT  : pred-chunk Square+accumulate (8x, emission interleaved into the mc
         loop so per-mc Sqrt(|cand|^2) ops are not stuck behind them), plus
         the two tiny Sqrts.
  DVE  : per mc: diag extract (mask-mult + reduce), reciprocal, then the
         psum-freeing drain osb = psum * (1/|cand|) (per-partition scalar).
         The second scale (* broadcast 1/|pred| row, bf16 out) is deferred
         DEFER iterations so its wait on the broadcast never stalls the DVE
         queue between psum drains.
  Pool : one partition_broadcast (1/|pred| row).
  DMA  : input loads ordered aT, bT0-2, pn x8 (split), bT3-15 to match PE
         consumption order; bf16 stores issued from SP/HWDGE.
"""

import numpy as np

import concourse.bacc as bacc
import concourse.mybir as mybir
import concourse.tile as tile
from concourse.bass_utils import run_bass_kernel_spmd

P = 128
N = 1024  # fp_pred rows
K = 4096  # feature dim
M_FULL = 16384  # fp_cand rows
N_CORES = 8
M = M_FULL // N_CORES  # 2048 cand rows per core
NB = N // P  # 8 pred row-chunks
MC = M // P  # 16 cand chunks per core
KK = K // 256  # 16 DoubleRow contraction super-chunks
FREE = 512  # moving free dim / psum bank width
DEFER = 11  # mc-iterations to defer the 1/|pred| multiply + store

F32 = mybir.dt.float32
BF16 = mybir.dt.bfloat16
FP8 = mybir.dt.float8e4
F8NP = mybir.dt.np(FP8)
AF = mybir.ActivationFunctionType
DR = mybir.MatmulPerfMode.DoubleRow

# pred-square chunk -> mc iteration in whose slot it is emitted on ACT
SQ_SLOT = {0: -1, 1: -1, 2: -1, 3: 2, 4: 3, 5: 4, 6: 5, 7: 6}
INVA_SLOT = 8  # mc iteration that emits the 1/|pred| round-trip + broadcast

_compiled = None


def _dedup_ldweights(nc):
    """Drop consecutive identical InstLdweights (the bass tile pipeline
    emits one per matmul; our inner loop issues 3 matmuls per stationary).
    Dangling dependency references are remapped to the surviving load."""
    renames = {}

    def key(i):
        return (repr(i.ins[0]), str(i.perf_mode), str(i.tile_position))

    for blk in nc.main_func.blocks:
        last = None
        drop = []
        for i in blk.instructions:
            if isinstance(i, mybir.InstLdweights):
                k = key(i)
                if last is not None and key(last) == k:
                    renames[i.name] = last.name
                    drop.append(i)
                else:
                    last = i
            elif isinstance(i, mybir.InstMatmult):
                pass
            elif getattr(i, "engine", None) == mybir.EngineType.PE:
                last = None
        for d in drop:
            blk.instructions.remove(d)
    if renames:
        for blk in nc.main_func.blocks:
            for i in blk.instructions:
                i.remap_dependency_names(renames)


def _build(repeats=1, abl=()):
    nc = bacc.Bacc(None, target_bir_lowering=False)
    at8 = nc.dram_tensor("at8", (P, KK, 2, N), FP8, kind="ExternalInput")
    bt8 = nc.dram_tensor("bt8", (P, MC, KK, 2, P), FP8, kind="ExternalInput")
    pn8 = nc.dram_tensor("pn8", (P, NB, K), FP8, kind="ExternalInput")
    msk = nc.dram_tensor("dmask", (P, P), BF16, kind="ExternalInput")
    out = nc.dram_tensor("scores_t", (M, N), BF16, kind="ExternalOutput")

    with tile.TileContext(nc) as tc:
        with (
            tc.tile_pool(name="dram", bufs=1, space="DRAM") as dram_pool,
            tc.tile_pool(name="big", bufs=1) as big_pool,
            tc.tile_pool(name="sq", bufs=1) as sq_pool,
            tc.tile_pool(name="small", bufs=2) as small_pool,
            tc.tile_pool(name="bc", bufs=1) as bc_pool,
            tc.tile_pool(name="outp", bufs=DEFER + 2) as out_pool,
            tc.tile_pool(name="outbf", bufs=3) as outbf_pool,
            tc.tile_pool(name="ps", bufs=6, space="PSUM") as ps_pool,
            tc.tile_pool(name="psd", bufs=2, space="PSUM") as psd_pool,
        ):
            for rep in range(repeats):
                inva_dram = dram_pool.tile([N], F32, tag="invad", name=f"invad{rep}")
                pnat = big_pool.tile([P, NB, K], FP8, tag="pnat", name=f"pnat{rep}")
                # aT double-buffered: lets iteration r+1's pred reload start
                # before iteration r's last matmul has consumed the old copy
                aT = big_pool.tile(
                    [P, KK, 2, N], FP8, tag="aT", name=f"aT{rep}", bufs=2
                )
                bT = big_pool.tile([P, MC, KK, 2, P], FP8, tag="bT", name=f"bT{rep}")
                mask = small_pool.tile([P, P], BF16, tag="mask", name=f"mask{rep}")

                # input loads in PE-consumption order; pred-natural slices
                # interleaved into the bT stream where DMA has slack
                nc.sync.dma_start(mask[:], msk[:])
                nc.sync.dma_start(aT[:], at8[:])
                for mc in range(3):
                    nc.sync.dma_start(bT[:, mc], bt8[:, mc])
                if "nopred" not in abl:
                    nc.sync.dma_start(pnat[:, 0:4], pn8[:, 0:4])
                for mc in range(3, 7):
                    nc.sync.dma_start(bT[:, mc], bt8[:, mc])
                if "nopred" not in abl:
                    nc.sync.dma_start(pnat[:, 4:NB], pn8[:, 4:NB])
                for mc in range(7, MC):
                    nc.sync.dma_start(bT[:, mc], bt8[:, mc])

                # ---- norm-result tiles ----
                nsq_a = small_pool.tile([P, NB], F32, tag="nsqa", name=f"nsqa{rep}")
                nrm_a = small_pool.tile([P, NB], F32, tag="nrma", name=f"nrma{rep}")
                inv_a = small_pool.tile([P, NB], F32, tag="inva", name=f"inva{rep}")
                nsq_b = small_pool.tile([P, MC], F32, tag="nsqb", name=f"nsqb{rep}")
                nrm_b = small_pool.tile([P, MC], F32, tag="nrmb", name=f"nrmb{rep}")
                inv_b = small_pool.tile([P, MC], F32, tag="invb", name=f"invb{rep}")
                inva_bc = bc_pool.tile([P, N], F32, tag="invbc", name=f"invbc{rep}")

                def act_square(nb):
                    sq = sq_pool.tile([P, K], FP8, tag="sq", name=f"sq{rep}_{nb}")
                    nc.scalar.activation(
                        sq[:], pnat[:, nb], AF.Square,
                        accum_out=nsq_a[:, nb : nb + 1],
                    )

                for nb, slot in SQ_SLOT.items():
                    if slot < 0 and "nopred" not in abl:
                        act_square(nb)

                # deferred-drain state: list of (mc, [osb_h0, osb_h1])
                pending = []

                def emit_second_scale(mc, osbs):
                    # ob = (osb * |cand_mc|) * (1/|pred|) — the |cand| factor
                    # corrects the 1/|cand|^2 used by the psum-freeing drain
                    for h, osb in enumerate(osbs):
                        ob = outbf_pool.tile(
                            [P, FREE], BF16, tag="obf", name=f"obf{rep}_{mc}_{h}"
                        )
                        if "nopred" in abl and "nocand" in abl:
                            nc.vector.tensor_copy(ob[:], osb[:])
                        elif "nopred" in abl:
                            nc.vector.tensor_scalar_mul(
                                ob[:], osb[:], nrm_b[:, mc : mc + 1]
                            )
                        elif "nocand" in abl:
                            nc.vector.tensor_tensor(
                                ob[:], osb[:],
                                inva_bc[:, h * FREE : (h + 1) * FREE],
                                mybir.AluOpType.mult,
                            )
                        else:
                            nc.vector.scalar_tensor_tensor(
                                ob[:], osb[:], nrm_b[:, mc : mc + 1],
                                inva_bc[:, h * FREE : (h + 1) * FREE],
                                mybir.AluOpType.mult, mybir.AluOpType.mult,
                            )
                        nc.sync.dma_start(
                            out[mc * P : (mc + 1) * P, h * FREE : (h + 1) * FREE],
                            ob[:],
                        )

                for mc in range(MC):
                    psd = psd_pool.tile(
                        [P, FREE], F32, tag="psd", name=f"psd{rep}_{mc}"
                    )
                    ps0 = ps_pool.tile([P, FREE], F32, tag="ps", name=f"ps{rep}_{mc}_0")
                    ps1 = ps_pool.tile([P, FREE], F32, tag="ps", name=f"ps{rep}_{mc}_1")
                    for kk in range(KK):
                        st, sp = kk == 0, kk == KK - 1
                        w = bT[:, mc, kk]  # [P, 2, P] stationary
                        nc.tensor.matmul(
                            psd[:, :P], w, w, start=st, stop=sp, perf_mode=DR
                        )
                        nc.tensor.matmul(
                            ps0[:], w, aT[:, kk, :, 0:FREE],
                            start=st, stop=sp, perf_mode=DR,
                        )
                        nc.tensor.matmul(
                            ps1[:], w, aT[:, kk, :, FREE:N],
                            start=st, stop=sp, perf_mode=DR,
                        )
                    # cand norms for this chunk, all-DVE on the psum-critical
                    # path: Gram diag -> 1/|cand|^2 (no sqrt round-trip; the
                    # deferred stage multiplies |cand| back in)
                    if "nocand" not in abl:
                        tmp = small_pool.tile(
                            [P, P], F32, tag="dtmp", name=f"dtmp{rep}_{mc}"
                        )
                        nc.vector.tensor_tensor(
                            tmp[:], psd[:, :P], mask[:], mybir.AluOpType.mult
                        )
                        nc.vector.tensor_reduce(
                            nsq_b[:, mc : mc + 1], tmp[:],
                            mybir.AxisListType.X, mybir.AluOpType.add,
                        )
                        nc.vector.reciprocal(
                            inv_b[:, mc : mc + 1], nsq_b[:, mc : mc + 1]
                        )
                        # |cand| for the deferred correction (ACT, off-path)
                        nc.scalar.activation(
                            nrm_b[:, mc : mc + 1], nsq_b[:, mc : mc + 1], AF.Sqrt
                        )
                    # stage-1 drain: frees the psum banks without waiting on
                    # anything but this chunk's own 1/|cand|^2
                    osbs = []
                    for h, ps in ((0, ps0), (1, ps1)):
                        osb = out_pool.tile(
                            [P, FREE], F32, tag="osb", name=f"osb{rep}_{mc}_{h}"
                        )
                        if "nocand" in abl:
                            nc.vector.tensor_copy(osb[:], ps[:])
                        else:
                            nc.vector.tensor_scalar_mul(
                                osb[:], ps[:], inv_b[:, mc : mc + 1]
                            )
                        osbs.append(osb)
                    pending.append((mc, osbs))

                    # interleaved ACT work for the pred-norm path
                    for nb, slot in SQ_SLOT.items():
                        if slot == mc and "nopred" not in abl:
                            act_square(nb)
                    if mc == INVA_SLOT and "nopred" not in abl:
                        nc.scalar.activation(nrm_a[:], nsq_a[:], AF.Sqrt)
                        nc.vector.reciprocal(inv_a[:], nrm_a[:])
                        nc.sync.dma_start(
                            inva_dram[:].rearrange("(nb p) -> p nb", p=P), inv_a[:]
                        )
                        row = small_pool.tile(
                            [1, N], F32, tag="invrow", name=f"invrow{rep}"
                        )
                        nc.sync.dma_start(row[:], inva_dram[None, :])
                        nc.gpsimd.partition_broadcast(inva_bc[:], row[:])

                    # stage-2 drain, deferred DEFER iterations
                    if len(pending) > DEFER:
                        emit_second_scale(*pending.pop(0))
                for args in pending:
                    emit_second_scale(*args)
    _dedup_ldweights(nc)
    nc.compile()
    return nc


def _get_compiled():
    global _compiled
    if _compiled is None:
        _compiled = _build()
    return _compiled


def prepare_inputs(fp_pred: np.ndarray, fp_cand: np.ndarray) -> list[dict]:
    """Quantize to fp8 once and marshal into the device layouts, per core."""
    predq = np.asarray(fp_pred, dtype=np.float32).astype(F8NP)
    candq = np.asarray(fp_cand, dtype=np.float32).astype(F8NP)
    assert predq.shape == (N, K) and candq.shape == (M_FULL, K)

    # at8[p, kk, i, n] = predq[n, (2kk+i)*128+p]
    at = np.ascontiguousarray(predq.T.reshape(KK, 2, P, N).transpose(2, 0, 1, 3))
    # pn8[p, nb, k] = predq[nb*128+p, k]
    pn = np.ascontiguousarray(predq.reshape(NB, P, K).transpose(1, 0, 2))
    mask = np.eye(P, dtype=mybir.dt.np(BF16))

    in_maps = []
    for c in range(N_CORES):
        cq = candq[c * M : (c + 1) * M]
        # bt8[p, mc, kk, i, m'] = cq[mc*128+m', (2kk+i)*128+p]
        bt = np.ascontiguousarray(
            cq.T.reshape(KK, 2, P, MC, P).transpose(2, 3, 0, 1, 4)
        )
        in_maps.append({"at8": at, "bt8": bt, "pn8": pn, "dmask": mask})
    return in_maps


def kernel(fp_pred: np.ndarray, fp_cand: np.ndarray) -> np.ndarray:
    nc = _get_compiled()
    in_maps = prepare_inputs(fp_pred, fp_cand)
    res = run_bass_kernel_spmd(nc, in_maps, core_ids=list(range(N_CORES)))
    full = np.empty((N, M_FULL), dtype=np.float32)
    for c in range(N_CORES):
        full[:, c * M : (c + 1) * M] = res.results[c]["scores_t"].T.astype(np.float32)
    return full
